# revision 22
# baseline (speedup 1.0000x reference)
"""Causal self-attention kernel for 8 Trainium2 NeuronCores.

Sharding: core c -> (batch b = c // 2, head-group g = c % 2).
Each core computes attention for its batch over its 8 heads and a partial
output projection; the host sums the two head-group partials per batch and
adds b_proj.

Reference shapes: x [4, 2048, 1024], W_attn [1024, 3072], b_attn [3072],
W_proj [1024, 1024], b_proj [1024]; NH=16, HD=64.
"""

import numpy as np

import bass_rust
import concourse.bass as bass
import concourse.mybir as mybir
import concourse.tile as tile
from concourse.bass_utils import run_bass_kernel_spmd

DT = mybir.dt
AF = mybir.ActivationFunctionType
ALU = mybir.AluOpType

P = 128
T = 2048          # sequence length
CIN = 1024        # input channels
CL = 512          # local channels (8 heads x 64)
NHL = 8           # local heads
HD = 64
KT = CIN // P     # 8 contraction tiles for qkv
TT = T // P       # 16 t-tiles
IC = T // 512     # 4 i-chunks of 512
COUT = 1024       # proj output channels
SCALE = 1.0 / 8.0  # 1/sqrt(HD)
NEG = -30000.0    # additive causal mask (exp underflows to 0)


class PatchedTileContext(tile.TileContext):
    """Work around walrus's 1-sync-wait-per-Drain limit: split the final
    drain's waits across one Drain instruction per proc."""

    def _drain_and_barrier(self, tick_clock, wait_clock):
        ScopedClock = bass_rust.ScopedClock
        VectorClock = bass_rust.VectorClock
        ticks = eval(repr(tick_clock.global_clock).replace("VectorClock(", "").rstrip(")"))
        for p, t in [(p, t) for p, t in enumerate(ticks) if t > 0]:
            part = [0] * len(ticks)
            part[p] = t
            d = self.nc.sync.drain()
            wait_clock.add_sem_waits(d.ins, ScopedClock({None: VectorClock(part)}))
        self.nc.all_engine_barrier()
        popped = self.nc._tile_sem_poison_stack.pop()
        assert popped is self._sem_poison
        self.nc.clear_and_free_semaphores(list(self.sems.allocated().values()))
        self.nc.all_engine_barrier()


# Max sync-waits this walrus build encodes per instruction. SP pseudo-DMA /
# CTRL instructions take a single wait; excess waits move onto NoOps that
# stall the same engine immediately before the instruction.
_MAX_WAITS = {}
_MAX_WAITS_DEFAULT = 1


def split_multi_waits(nc):
    for fn in nc.m.functions:
        for blk in fn.blocks:
            insts = blk.instructions
            out = []
            for inst in insts:
                si = getattr(inst, "sync_info", None)
                waits = list(si.on_wait) if si is not None and si.on_wait else []
                cap = _MAX_WAITS.get(str(inst.opcode), _MAX_WAITS_DEFAULT)
                if len(waits) > cap:
                    extra, keep = waits[:-cap], waits[-cap:]
                    for k, w in enumerate(extra):
                        nn = mybir.InstNoOp(name=f"{inst.name}-w{k}", ins=[], outs=[])
                        nn.engine = inst.engine
                        nn.sync_info = bass_rust.SyncInfo(on_wait=[w], on_update=[])
                        out.append(nn)
                    inst.sync_info = bass_rust.SyncInfo(
                        on_wait=keep, on_update=list(si.on_update or []))
                out.append(inst)
            blk.instructions = out


def act_reciprocal(nc, out, in_):
    """ACT-table reciprocal (bypasses the bass accuracy guard; tolerance here
    is loose enough)."""
    eng = nc.scalar
    inputs = [
        eng.lower_ap(in_),
        mybir.ImmediateValue(dtype=DT.float32, value=0.0),
        mybir.ImmediateValue(dtype=DT.float32, value=1.0),
        mybir.ImmediateValue(dtype=DT.float32, value=0.0),
    ]
    return eng.add_instruction(mybir.InstActivation(
        name=nc.get_next_instruction_name(),
        func=AF.Reciprocal,
        ins=inputs,
        outs=[eng.lower_ap(out)],
    ))


def build_program(split_waits=True):
    nc = bass.Bass()
    x_d = nc.dram_tensor("x", [T, CIN], DT.float32, kind="ExternalInput")
    wqk_d = nc.dram_tensor("wqk", [CIN, 2 * CL], DT.float32, kind="ExternalInput")
    wv_d = nc.dram_tensor("wv", [CIN, CL], DT.float32, kind="ExternalInput")
    bqk_d = nc.dram_tensor("bqk", [2 * CL], DT.float32, kind="ExternalInput")
    bv_d = nc.dram_tensor("bv", [CL], DT.float32, kind="ExternalInput")
    wp_d = nc.dram_tensor("wp", [CL, COUT], DT.float32, kind="ExternalInput")
    out_d = nc.dram_tensor("out", [T, COUT], DT.float32, kind="ExternalOutput")

    with PatchedTileContext(nc) as tc:
        with (
            tc.tile_pool(name="const", bufs=1) as const,
            tc.tile_pool(name="big", bufs=1) as big,
            tc.tile_pool(name="stage", bufs=2) as stage,
            tc.tile_pool(name="xs", bufs=2) as xs_pool,
            tc.tile_pool(name="xb", bufs=3) as xb_pool,
            tc.tile_pool(name="pt", bufs=10) as pt_pool,
            tc.tile_pool(name="ps_mm", bufs=2, space="PSUM") as ps_mm,
            tc.tile_pool(name="ps_qk", bufs=2, space="PSUM") as ps_qk,
            tc.tile_pool(name="ps_y", bufs=2, space="PSUM") as ps_y,
        ):
            # psum: mm [128,1024]x2 (4 banks) + qk [128,512]x2 + y x2 = 8
            def mm_tile():
                return ps_mm.tile([P, 1024], DT.float32, tag="mm", name="mmt")

            def qk_tile():
                return ps_qk.tile([P, 512], DT.float32, tag="qkp", name="qkp")

            # ---- constants ----
            ones1 = const.tile([65, P], DT.bfloat16, tag="ones1")
            nc.gpsimd.memset(ones1[:], 1.0)

            ident_bf = const.tile([P, P], DT.bfloat16, tag="ident")
            from concourse.masks import make_identity
            make_identity(nc, ident_bf[:])

            # causal mask for diagonal 128-col sub-blocks (d-independent):
            # mask[p, i] = 0 if i - p >= 0 else NEG
            masks = const.tile([P, 1, P], DT.float32, tag="masks")
            nc.gpsimd.memset(masks[:], 0.0)
            nc.gpsimd.affine_select(
                out=masks[:, 0, :],
                in_=masks[:, 0, :],
                compare_op=ALU.is_ge,
                fill=NEG,
                base=0,
                pattern=[[1, P]],
                channel_multiplier=-1,
            )

            # biases: bqk as [128, 8] per-partition layout (c_out on partitions)
            bqk_sb = const.tile([P, 2 * CL // P], DT.float32, tag="bqk")
            nc.sync.dma_start(bqk_sb[:], bqk_d.rearrange("(mt p) -> p mt", p=P))
            # bv_sb[64t+p, hp] = bv[64(2hp+t)+p]: head pair hp stacked on 128
            bv_sb = const.tile([P, NHL // 2], DT.float32, tag="bv")
            nc.sync.dma_start(
                bv_sb[:], bv_d.rearrange("(hp t p) -> (t p) hp", t=2, p=HD))

            # preload the exp ACT table during the prologue
            dummy = const.tile([1, 2], DT.float32, tag="dummy")
            nc.gpsimd.memset(dummy[:], 0.0)
            nc.scalar.activation(dummy[:], dummy[:], AF.Exp)

            # ---- x: DMA f32 -> DVE cast bf16 -> DMA-xbar transpose ----
            # xT[p, tt, ko, j] = x[128*tt + j, 128*ko + p]
            xT = big.tile([P, TT, KT, P], DT.bfloat16, tag="xT")
            x_r = x_d.rearrange("(tt p) c -> p tt c", p=P)
            qkT_bf = big.tile([P, KT, T], DT.bfloat16, tag="qkT_bf")
            v_sb = big.tile([P, TT, NHL, HD + 1], DT.bfloat16, tag="v_sb")
            nc.gpsimd.memset(v_sb[:, :, :, HD], 1.0)

            def load_x(tt):
                e1 = nc.sync if tt % 2 == 0 else nc.scalar
                xst = xs_pool.tile([P, CIN], DT.float32, tag="xstage")
                e1.dma_start(xst[:], x_r[:, tt, :])
                xbt = xb_pool.tile([P, CIN], DT.bfloat16, tag="xbstage")
                nc.vector.tensor_copy(xbt[:], xst[:])
                # 8 PE transposes into one psum tile (bf16 view), 1 copy out
                pst = mm_tile()
                pbf = pst[:].bitcast(DT.bfloat16)
                for ko in range(KT):
                    nc.tensor.transpose(
                        pbf[:, ko * P:(ko + 1) * P],
                        xbt[:, ko * P:(ko + 1) * P], ident_bf[:])
                nc.vector.tensor_copy(
                    xT[:, tt, :, :],
                    pbf[:, 0:CIN].rearrange("p (ko j) -> p ko j", ko=KT))

            for tt in range(4):
                load_x(tt)

            # ---- qkT = (x @ Wqk)^T in [c, t] layout; v in [t, c] layout ----
            # ---- weights: load fp32, cast to bf16 on DVE ----
            wqk_bf = big.tile([P, KT, 2 * CL], DT.bfloat16, tag="wqk_bf")
            wv_bf = big.tile([P, KT, CL], DT.bfloat16, tag="wv_bf")
            wqk_r = wqk_d.rearrange("(ko p) n -> p ko n", p=P)
            wv_r = wv_d.rearrange("(ko p) n -> p ko n", p=P)
            for ko in range(KT):
                stv = stage.tile([P, 2 * CL], DT.float32, tag="wstage", name="stv")[:, 0:CL]
                nc.scalar.dma_start(stv[:], wv_r[:, ko, :])
                nc.vector.tensor_copy(wv_bf[:, ko, :], stv[:])
                st = stage.tile([P, 2 * CL], DT.float32, tag="wstage", name="st")
                nc.scalar.dma_start(st[:], wqk_r[:, ko, :])
                nc.vector.tensor_copy(wqk_bf[:, ko, :], st[:])
            wp_bf = big.tile([P, CL // P, COUT], DT.bfloat16, tag="wp_bf")
            wp_r = wp_d.rearrange("(ko p) n -> p ko n", p=P)
            for ko in range(CL // P):
                stp = stage.tile([P, 2 * CL], DT.float32, tag="wstage", name="stp")[:, 0:COUT]
                nc.scalar.dma_start(stp[:], wp_r[:, ko, :])
                nc.gpsimd.tensor_copy(wp_bf[:, ko, :], stp[:])

            # qk matmul chain for one (c_out tile, t chunk); bias on DVE.
            # Generator form emits one instruction per next() so chains can
            # be woven between attention blocks at matmul granularity.
            def qk_chain_steps(mi, nic):
                pq = qk_tile()
                for ki in range(KT):
                    nc.tensor.matmul(
                        pq[:],
                        wqk_bf[:, ki, mi * P:(mi + 1) * P],
                        xT[:, 4 * nic:4 * nic + 4, ki, :],
                        start=(ki == 0), stop=(ki == KT - 1),
                    )
                    yield
                if mi < 4:
                    # q: (psum + bias) * SCALE
                    nc.vector.tensor_scalar(
                        qkT_bf[:, mi, nic * 512:(nic + 1) * 512],
                        pq[:], bqk_sb[:, mi:mi + 1], SCALE,
                        ALU.add, ALU.mult,
                    )
                else:
                    nc.vector.tensor_scalar_add(
                        qkT_bf[:, mi, nic * 512:(nic + 1) * 512],
                        pq[:], bqk_sb[:, mi:mi + 1],
                    )
                yield

            def qk_chain(mi, nic):
                for _ in qk_chain_steps(mi, nic):
                    pass

            # prologue: v for all heads + q,k for head-pair 0 only; the q,k
            # chains for hp+1 are interleaved into hp's attention below so the
            # PE stays busy while ACT runs the softmax exps.
            for nic in range(T // 512):
                if 4 * nic + 4 < TT:
                    for tt in range(4 * nic + 4, min(4 * nic + 8, TT)):
                        load_x(tt)
                # v = x @ Wv in [t, c] layout (first 8 t-tiles; rest are
                # interleaved into hp0's attention as PE filler)
                for tt in range(4 * nic, 4 * nic + 4) if nic < 2 else ():
                    pv = qk_tile()
                    for ki in range(KT):
                        nc.tensor.matmul(
                            pv[:],
                            xT[:, tt, ki, :],
                            wv_bf[:, ki, :],
                            start=(ki == 0), stop=(ki == KT - 1),
                        )
                    nc.vector.tensor_copy(
                        v_sb[:, tt, :, 0:HD],
                        pv[:].rearrange("p (h e) -> p h e", h=NHL),
                    )
                for mi in (0, 4, 1, 5):
                    qk_chain(mi, nic)

            # ---- attention, head-pair packed, software-pipelined ----
            # Heads 2hp (partitions 0:64) and 2hp+1 (64:128) run as one
            # stream: S matmuls pack into row groups 0-1 / 2-3 concurrently,
            # one Exp covers both heads, PV lags LAG j-tiles behind S.
            # qk chains for hp+1 fill the PE while ACT exps hp; each hp's
            # normalize tail (1/l via ln+exp, same ACT table set) overlaps
            # the next hp's attention.
            yT_bf = big.tile([P, CL // P, T], DT.bfloat16, tag="yT_bf")
            out_r = out_d.rearrange("(tt p) c -> p tt c", p=P)

            def proj_tt(tt):
                pp = mm_tile()
                for oc in range(COUT // 512):
                    for ci in range(CL // P):
                        nc.tensor.matmul(
                            pp[:, oc * 512:(oc + 1) * 512],
                            yT_bf[:, ci, tt * P:(tt + 1) * P],
                            wp_bf[:, ci, oc * 512:(oc + 1) * 512],
                            start=(ci == 0), stop=(ci == CL // P - 1),
                        )
                ot = stage.tile([P, 2 * CL], DT.float32, tag="wstage",
                                name="ot")[:, 0:1024]
                if tt % 2 == 0:
                    nc.vector.tensor_copy(ot[:], pp[:])
                else:
                    nc.scalar.copy(ot[:], pp[:])
                nc.sync.dma_start(out_r[:, tt, :], ot[:])
            # l rows stored at partition bases {0,32,64} (matmul-rhs legal)
            l_buf = big.tile([65, 11, 512], DT.bfloat16, tag="l_buf")
            r_bf = big.tile([65, 11, 512], DT.bfloat16, tag="r_bf")
            ust = const.tile([65, 4, 512], DT.float32, tag="ust")
            def v_chain_steps(tt):
                pv = qk_tile()
                for ki in range(KT):
                    nc.tensor.matmul(
                        pv[:],
                        xT[:, tt, ki, :],
                        wv_bf[:, ki, :],
                        start=(ki == 0), stop=(ki == KT - 1),
                    )
                    yield
                nc.vector.tensor_copy(
                    v_sb[:, tt, :, 0:HD],
                    pv[:].rearrange("p (h e) -> p h e", h=NHL),
                )
                yield

            def global_filler():
                for tt in range(8, TT):
                    yield from v_chain_steps(tt)
                for mi in (2, 6, 3, 7):
                    for nic in range(4):
                        yield from qk_chain_steps(mi, nic)

            LAG = 4
            pending_tail = None
            filler = global_filler()
            for hp in range(NHL // 2):
                hA, hB = 2 * hp, 2 * hp + 1
                qt, kt_i = hp, 4 + hp
                for ic in range(IC):
                    jt_max = 4 * ic + 3
                    pyA = ps_y.tile([HD + 1, 512], DT.float32, tag="y", name="pyA")
                    pyB = ps_y.tile([HD + 1, 512], DT.float32, tag="y", name="pyB")
                    pts = []
                    offs = []

                    def emit_pv(jt):
                        pt = pts[jt]
                        o = offs[jt]
                        nc.tensor.matmul(
                            pyA[:, o:512], v_sb[:, jt, hA, :], pt[:, o:512],
                            start=(jt == 0), stop=(jt == jt_max))
                        nc.tensor.matmul(
                            pyB[:, o:512], v_sb[:, jt, hB, :],
                            pt[:, 512 + o:1024],
                            start=(jt == 0), stop=(jt == jt_max))

                    for jt in range(jt_max + 1):
                        d = jt - 4 * ic
                        off = 128 * d if d > 0 else 0
                        ps = mm_tile()
                        isl = slice(ic * 512 + off, (ic + 1) * 512)
                        nc.tensor.matmul(
                            ps[:, off:512],
                            qkT_bf[0:HD, kt_i, jt * P:(jt + 1) * P],
                            qkT_bf[0:HD, qt, isl],
                            start=True, stop=True)
                        nc.tensor.matmul(
                            ps[:, 512 + off:1024],
                            qkT_bf[HD:P, kt_i, jt * P:(jt + 1) * P],
                            qkT_bf[HD:P, qt, isl],
                            start=True, stop=True)
                        ps2 = ps[:].rearrange("p (g x) -> p g x", g=2)
                        pt = pt_pool.tile([P, 1024], DT.bfloat16, tag="pt")
                        pt2 = pt[:].rearrange("p (g x) -> p g x", g=2)
                        if d >= 0:
                            # mask only the diagonal 128-col sub-block
                            nc.vector.tensor_tensor(
                                ps2[:, :, off:off + P], ps2[:, :, off:off + P],
                                masks[:, 0:1, :].to_broadcast((P, 2, P)),
                                ALU.add)
                            nc.scalar.activation(
                                pt2[:, :, off:512], ps2[:, :, off:512], AF.Exp)
                        else:
                            nc.scalar.activation(pt[:], ps[:], AF.Exp)
                        pts.append(pt)
                        offs.append(off)
                        if jt >= LAG:
                            emit_pv(jt - LAG)
                        for _ in range(4):
                            next(filler, None)
                    for jt in range(max(0, jt_max + 1 - LAG), jt_max + 1):
                        emit_pv(jt)
                    # stash unnormalized z into yT (both heads at once); l rows
                    idxA, idxB = hA * IC + ic, hB * IC + ic
                    nc.vector.tensor_copy(
                        yT_bf[0:HD, hp, ic * 512:(ic + 1) * 512], pyA[0:HD, :])
                    nc.vector.tensor_copy(
                        yT_bf[HD:P, hp, ic * 512:(ic + 1) * 512], pyB[0:HD, :])
                    nc.vector.tensor_copy(
                        l_buf[32 * (idxA % 3):32 * (idxA % 3) + 1, idxA // 3, :],
                        pyA[HD:HD + 1, :])
                    nc.vector.tensor_copy(
                        l_buf[32 * (idxB % 3):32 * (idxB % 3) + 1, idxB // 3, :],
                        pyB[HD:HD + 1, :])
                    if ic == 0 and pending_tail is not None:
                        pending_tail()
                        pending_tail = None


                def make_tail(h, with_proj=False):
                    def tail():
                        # r = exp(-ln(l)): same ACT table set, no swap
                        c0 = (8 * h) // 3
                        ncol = (8 * h + 7) // 3 - c0 + 1
                        nc.scalar.activation(
                            ust[:, 0:ncol, :], l_buf[:, c0:c0 + ncol, :],
                            AF.Ln)
                        nc.scalar.activation(
                            r_bf[:, c0:c0 + ncol, :], ust[:, 0:ncol, :],
                            AF.Exp, scale=-1.0)
                        thA, thB = 2 * h, 2 * h + 1
                        for tic in range(IC):
                            idxA, idxB = thA * IC + tic, thB * IC + tic
                            pbt = qk_tile()
                            bA, bB = 32 * (idxA % 3), 32 * (idxB % 3)
                            nc.tensor.matmul(
                                pbt[0:HD, 0:512], ones1[bA:bA + 1, 0:HD],
                                r_bf[bA:bA + 1, idxA // 3, :],
                                start=True, stop=True)
                            nc.tensor.matmul(
                                pbt[HD:P, 0:512], ones1[bB:bB + 1, 0:HD],
                                r_bf[bB:bB + 1, idxB // 3, :],
                                start=True, stop=True, tile_position=(bB, HD))
                            ysl = yT_bf[:, h, tic * 512:(tic + 1) * 512]
                            nc.vector.tensor_mul(ysl, ysl, pbt[:])
                            nc.vector.tensor_scalar_add(
                                ysl, ysl, bv_sb[:, h:h + 1])
                            if with_proj:
                                for tt in range(4 * tic, 4 * tic + 4):
                                    proj_tt(tt)
                    return tail
                pending_tail = make_tail(hp, with_proj=(hp == NHL // 2 - 1))

            pending_tail()
    if split_waits:
        split_multi_waits(nc)
    return nc


_PROGRAM = None


def _get_program():
    global _PROGRAM
    if _PROGRAM is None:
        _PROGRAM = build_program()
    return _PROGRAM


def _make_in_maps(x, W_attn, b_attn, W_proj):
    x = np.asarray(x, dtype=np.float32)
    W_attn = np.asarray(W_attn, dtype=np.float32)
    b_attn = np.asarray(b_attn, dtype=np.float32)
    W_proj = np.asarray(W_proj, dtype=np.float32)
    in_maps = []
    for c in range(8):
        b, g = divmod(c, 2)
        sl = slice(CL * g, CL * (g + 1))
        wq = W_attn[:, 0:1024][:, sl]
        wk = W_attn[:, 1024:2048][:, sl]
        wv = W_attn[:, 2048:3072][:, sl]
        bq = b_attn[0:1024][sl]
        bk = b_attn[1024:2048][sl]
        bv = b_attn[2048:3072][sl]
        in_maps.append({
            "x": np.ascontiguousarray(x[b]),
            "wqk": np.ascontiguousarray(np.concatenate([wq, wk], axis=1)),
            "wv": np.ascontiguousarray(wv),
            "bqk": np.ascontiguousarray(np.concatenate([bq, bk])),
            "bv": np.ascontiguousarray(bv),
            "wp": np.ascontiguousarray(W_proj[sl]),
        })
    return in_maps


def kernel(x, W_attn, b_attn, W_proj, b_proj, _trace_dir=None):
    nc = _get_program()
    in_maps = _make_in_maps(x, W_attn, b_attn, W_proj)
    kwargs = {}
    if _trace_dir is not None:
        kwargs = dict(trace=True, tmpdir=_trace_dir)
    res = run_bass_kernel_spmd(nc, in_maps, core_ids=list(range(8)), **kwargs)
    b_proj = np.asarray(b_proj, dtype=np.float32)
    out = np.empty((4, T, COUT), dtype=np.float32)
    for b in range(4):
        out[b] = res.results[2 * b]["out"] + res.results[2 * b + 1]["out"] + b_proj
    if _trace_dir is not None:
        kernel._last_exec_time_ns = res.exec_time_ns
        kernel._last_results = res
    return out



# revision 23
# speedup vs baseline: 1.0378x; 1.0378x over previous
"""Causal self-attention kernel for 8 Trainium2 NeuronCores.

Sharding: core c -> (batch b = c // 2, head-group g = c % 2).
Each core computes attention for its batch over its 8 heads and a partial
output projection; the host sums the two head-group partials per batch and
adds b_proj.

Reference shapes: x [4, 2048, 1024], W_attn [1024, 3072], b_attn [3072],
W_proj [1024, 1024], b_proj [1024]; NH=16, HD=64.
"""

import numpy as np

import bass_rust
import concourse.bass as bass
import concourse.mybir as mybir
import concourse.tile as tile
from concourse.bass_utils import run_bass_kernel_spmd

DT = mybir.dt
AF = mybir.ActivationFunctionType
ALU = mybir.AluOpType

P = 128
T = 2048          # sequence length
CIN = 1024        # input channels
CL = 512          # local channels (8 heads x 64)
NHL = 8           # local heads
HD = 64
KT = CIN // P     # 8 contraction tiles for qkv
TT = T // P       # 16 t-tiles
IC = T // 512     # 4 i-chunks of 512
COUT = 1024       # proj output channels
SCALE = 1.0 / 8.0  # 1/sqrt(HD)
NEG = -30000.0    # additive causal mask (exp underflows to 0)


class PatchedTileContext(tile.TileContext):
    """Work around walrus's 1-sync-wait-per-Drain limit: split the final
    drain's waits across one Drain instruction per proc."""

    def _drain_and_barrier(self, tick_clock, wait_clock):
        ScopedClock = bass_rust.ScopedClock
        VectorClock = bass_rust.VectorClock
        ticks = eval(repr(tick_clock.global_clock).replace("VectorClock(", "").rstrip(")"))
        for p, t in [(p, t) for p, t in enumerate(ticks) if t > 0]:
            part = [0] * len(ticks)
            part[p] = t
            d = self.nc.sync.drain()
            wait_clock.add_sem_waits(d.ins, ScopedClock({None: VectorClock(part)}))
        self.nc.all_engine_barrier()
        popped = self.nc._tile_sem_poison_stack.pop()
        assert popped is self._sem_poison
        self.nc.clear_and_free_semaphores(list(self.sems.allocated().values()))
        self.nc.all_engine_barrier()


# Max sync-waits this walrus build encodes per instruction. SP pseudo-DMA /
# CTRL instructions take a single wait; excess waits move onto NoOps that
# stall the same engine immediately before the instruction.
_MAX_WAITS = {}
_MAX_WAITS_DEFAULT = 1


def split_multi_waits(nc):
    for fn in nc.m.functions:
        for blk in fn.blocks:
            insts = blk.instructions
            out = []
            for inst in insts:
                si = getattr(inst, "sync_info", None)
                waits = list(si.on_wait) if si is not None and si.on_wait else []
                cap = _MAX_WAITS.get(str(inst.opcode), _MAX_WAITS_DEFAULT)
                if len(waits) > cap:
                    extra, keep = waits[:-cap], waits[-cap:]
                    for k, w in enumerate(extra):
                        nn = mybir.InstNoOp(name=f"{inst.name}-w{k}", ins=[], outs=[])
                        nn.engine = inst.engine
                        nn.sync_info = bass_rust.SyncInfo(on_wait=[w], on_update=[])
                        out.append(nn)
                    inst.sync_info = bass_rust.SyncInfo(
                        on_wait=keep, on_update=list(si.on_update or []))
                out.append(inst)
            blk.instructions = out


def act_reciprocal(nc, out, in_):
    """ACT-table reciprocal (bypasses the bass accuracy guard; tolerance here
    is loose enough)."""
    eng = nc.scalar
    inputs = [
        eng.lower_ap(in_),
        mybir.ImmediateValue(dtype=DT.float32, value=0.0),
        mybir.ImmediateValue(dtype=DT.float32, value=1.0),
        mybir.ImmediateValue(dtype=DT.float32, value=0.0),
    ]
    return eng.add_instruction(mybir.InstActivation(
        name=nc.get_next_instruction_name(),
        func=AF.Reciprocal,
        ins=inputs,
        outs=[eng.lower_ap(out)],
    ))


def build_program(split_waits=True):
    nc = bass.Bass()
    x_d = nc.dram_tensor("x", [T, CIN], DT.float32, kind="ExternalInput")
    wqk_d = nc.dram_tensor("wqk", [CIN, 2 * CL], DT.float32, kind="ExternalInput")
    wv_d = nc.dram_tensor("wv", [CIN, CL], DT.float32, kind="ExternalInput")
    bqk_d = nc.dram_tensor("bqk", [2 * CL], DT.float32, kind="ExternalInput")
    bv_d = nc.dram_tensor("bv", [CL], DT.float32, kind="ExternalInput")
    wp_d = nc.dram_tensor("wp", [CL, COUT], DT.float32, kind="ExternalInput")
    out_d = nc.dram_tensor("out", [T, COUT], DT.float32, kind="ExternalOutput")

    with PatchedTileContext(nc) as tc:
        with (
            tc.tile_pool(name="const", bufs=1) as const,
            tc.tile_pool(name="big", bufs=1) as big,
            tc.tile_pool(name="stage", bufs=2) as stage,
            tc.tile_pool(name="xs", bufs=2) as xs_pool,
            tc.tile_pool(name="xb", bufs=3) as xb_pool,
            tc.tile_pool(name="pt", bufs=10) as pt_pool,
            tc.tile_pool(name="ps_mm", bufs=2, space="PSUM") as ps_mm,
            tc.tile_pool(name="ps_qk", bufs=2, space="PSUM") as ps_qk,
            tc.tile_pool(name="ps_y", bufs=2, space="PSUM") as ps_y,
        ):
            # psum: mm [128,1024]x2 (4 banks) + qk [128,512]x2 + y x2 = 8
            def mm_tile():
                return ps_mm.tile([P, 1024], DT.float32, tag="mm", name="mmt")

            def qk_tile():
                return ps_qk.tile([P, 512], DT.float32, tag="qkp", name="qkp")

            # ---- constants ----
            ones1 = const.tile([65, P], DT.bfloat16, tag="ones1")
            nc.gpsimd.memset(ones1[:], 1.0)

            ident_bf = const.tile([P, P], DT.bfloat16, tag="ident")
            from concourse.masks import make_identity
            make_identity(nc, ident_bf[:])

            # causal mask for diagonal 128-col sub-blocks (d-independent):
            # mask[p, i] = 0 if i - p >= 0 else NEG
            masks = const.tile([P, 1, P], DT.float32, tag="masks")
            nc.gpsimd.memset(masks[:], 0.0)
            nc.gpsimd.affine_select(
                out=masks[:, 0, :],
                in_=masks[:, 0, :],
                compare_op=ALU.is_ge,
                fill=NEG,
                base=0,
                pattern=[[1, P]],
                channel_multiplier=-1,
            )

            # biases: bqk as [128, 8] per-partition layout (c_out on partitions)
            bqk_sb = const.tile([P, 2 * CL // P], DT.float32, tag="bqk")
            nc.sync.dma_start(bqk_sb[:], bqk_d.rearrange("(mt p) -> p mt", p=P))
            # bv_sb[64t+p, hp] = bv[64(2hp+t)+p]: head pair hp stacked on 128
            bv_sb = const.tile([P, NHL // 2], DT.float32, tag="bv")
            nc.sync.dma_start(
                bv_sb[:], bv_d.rearrange("(hp t p) -> (t p) hp", t=2, p=HD))

            # preload the exp ACT table during the prologue
            dummy = const.tile([1, 2], DT.float32, tag="dummy")
            nc.gpsimd.memset(dummy[:], 0.0)
            nc.scalar.activation(dummy[:], dummy[:], AF.Exp)

            # ---- x: DMA f32 -> DVE cast bf16 -> DMA-xbar transpose ----
            # xT[p, tt, ko, j] = x[128*tt + j, 128*ko + p]
            xT = big.tile([P, TT, KT, P], DT.bfloat16, tag="xT")
            x_r = x_d.rearrange("(tt p) c -> p tt c", p=P)
            qkT_bf = big.tile([P, KT, T], DT.bfloat16, tag="qkT_bf")
            v_sb = big.tile([P, TT, NHL, HD + 1], DT.bfloat16, tag="v_sb")
            nc.gpsimd.memset(v_sb[:, :, :, HD], 1.0)

            def load_x(tt):
                e1 = nc.sync if tt % 2 == 0 else nc.scalar
                xst = xs_pool.tile([P, CIN], DT.float32, tag="xstage")
                e1.dma_start(xst[:], x_r[:, tt, :])
                xbt = xb_pool.tile([P, CIN], DT.bfloat16, tag="xbstage")
                nc.vector.tensor_copy(xbt[:], xst[:])
                # 8 PE transposes into one psum tile (bf16 view), 1 copy out
                pst = mm_tile()
                pbf = pst[:].bitcast(DT.bfloat16)
                for ko in range(KT):
                    nc.tensor.transpose(
                        pbf[:, ko * P:(ko + 1) * P],
                        xbt[:, ko * P:(ko + 1) * P], ident_bf[:])
                nc.vector.tensor_copy(
                    xT[:, tt, :, :],
                    pbf[:, 0:CIN].rearrange("p (ko j) -> p ko j", ko=KT))

            for tt in range(4):
                load_x(tt)

            # ---- qkT = (x @ Wqk)^T in [c, t] layout; v in [t, c] layout ----
            # ---- weights: load fp32, cast to bf16 on DVE ----
            wqk_bf = big.tile([P, KT, 2 * CL], DT.bfloat16, tag="wqk_bf")
            wv_bf = big.tile([P, KT, CL], DT.bfloat16, tag="wv_bf")
            wqk_r = wqk_d.rearrange("(ko p) n -> p ko n", p=P)
            wv_r = wv_d.rearrange("(ko p) n -> p ko n", p=P)
            for ko in range(KT):
                stv = stage.tile([P, 2 * CL], DT.float32, tag="wstage", name="stv")[:, 0:CL]
                nc.scalar.dma_start(stv[:], wv_r[:, ko, :])
                nc.vector.tensor_copy(wv_bf[:, ko, :], stv[:])
                st = stage.tile([P, 2 * CL], DT.float32, tag="wstage", name="st")
                nc.scalar.dma_start(st[:], wqk_r[:, ko, :])
                nc.vector.tensor_copy(wqk_bf[:, ko, :], st[:])
            wp_bf = big.tile([P, CL // P, COUT], DT.bfloat16, tag="wp_bf")
            wp_r = wp_d.rearrange("(ko p) n -> p ko n", p=P)
            for ko in range(CL // P):
                stp = stage.tile([P, 2 * CL], DT.float32, tag="wstage", name="stp")[:, 0:COUT]
                nc.scalar.dma_start(stp[:], wp_r[:, ko, :])
                nc.gpsimd.tensor_copy(wp_bf[:, ko, :], stp[:])

            # qk matmul chain for one (c_out tile, t chunk); bias on DVE.
            # Generator form emits one instruction per next() so chains can
            # be woven between attention blocks at matmul granularity.
            def qk_chain_steps(mi, nic):
                pq = qk_tile()
                for ki in range(KT):
                    nc.tensor.matmul(
                        pq[:],
                        wqk_bf[:, ki, mi * P:(mi + 1) * P],
                        xT[:, 4 * nic:4 * nic + 4, ki, :],
                        start=(ki == 0), stop=(ki == KT - 1),
                    )
                    yield
                if mi < 4:
                    # q: (psum + bias) * SCALE
                    nc.vector.tensor_scalar(
                        qkT_bf[:, mi, nic * 512:(nic + 1) * 512],
                        pq[:], bqk_sb[:, mi:mi + 1], SCALE,
                        ALU.add, ALU.mult,
                    )
                else:
                    nc.vector.tensor_scalar_add(
                        qkT_bf[:, mi, nic * 512:(nic + 1) * 512],
                        pq[:], bqk_sb[:, mi:mi + 1],
                    )
                yield

            def qk_chain(mi, nic):
                for _ in qk_chain_steps(mi, nic):
                    pass

            # prologue: v for all heads + q,k for head-pair 0 only; the q,k
            # chains for hp+1 are interleaved into hp's attention below so the
            # PE stays busy while ACT runs the softmax exps.
            for nic in range(T // 512):
                if 4 * nic + 4 < TT:
                    for tt in range(4 * nic + 4, min(4 * nic + 8, TT)):
                        load_x(tt)
                # v = x @ Wv in [t, c] layout for the 4 t-tiles of this chunk
                for tt in range(4 * nic, 4 * nic + 4):
                    pv = qk_tile()
                    for ki in range(KT):
                        nc.tensor.matmul(
                            pv[:],
                            xT[:, tt, ki, :],
                            wv_bf[:, ki, :],
                            start=(ki == 0), stop=(ki == KT - 1),
                        )
                    nc.vector.tensor_copy(
                        v_sb[:, tt, :, 0:HD],
                        pv[:].rearrange("p (h e) -> p h e", h=NHL),
                    )
                for mi in (0, 4, 1, 5):
                    qk_chain(mi, nic)

            # ---- attention, head-pair packed, software-pipelined ----
            # Heads 2hp (partitions 0:64) and 2hp+1 (64:128) run as one
            # stream: S matmuls pack into row groups 0-1 / 2-3 concurrently,
            # one Exp covers both heads, PV lags LAG j-tiles behind S.
            # qk chains for hp+1 fill the PE while ACT exps hp; each hp's
            # normalize tail (1/l via ln+exp, same ACT table set) overlaps
            # the next hp's attention.
            yT_bf = big.tile([P, CL // P, T], DT.bfloat16, tag="yT_bf")
            out_r = out_d.rearrange("(tt p) c -> p tt c", p=P)

            def proj_tt(tt):
                pp = mm_tile()
                for oc in range(COUT // 512):
                    for ci in range(CL // P):
                        nc.tensor.matmul(
                            pp[:, oc * 512:(oc + 1) * 512],
                            yT_bf[:, ci, tt * P:(tt + 1) * P],
                            wp_bf[:, ci, oc * 512:(oc + 1) * 512],
                            start=(ci == 0), stop=(ci == CL // P - 1),
                        )
                ot = stage.tile([P, 2 * CL], DT.float32, tag="wstage",
                                name="ot")[:, 0:1024]
                if tt % 2 == 0:
                    nc.vector.tensor_copy(ot[:], pp[:])
                else:
                    nc.scalar.copy(ot[:], pp[:])
                nc.sync.dma_start(out_r[:, tt, :], ot[:])
            # l rows stored at partition bases {0,32,64} (matmul-rhs legal)
            l_buf = big.tile([65, 11, 512], DT.bfloat16, tag="l_buf")
            r_bf = big.tile([65, 11, 512], DT.bfloat16, tag="r_bf")
            ust = const.tile([65, 4, 512], DT.float32, tag="ust")
            LAG = 4
            pending_tail = None
            for hp in range(NHL // 2):
                hA, hB = 2 * hp, 2 * hp + 1
                qt, kt_i = hp, 4 + hp
                def make_filler(h):
                    for nic in range(4):
                        for mi in (h + 2, 6 + h):
                            yield from qk_chain_steps(mi, nic)
                filler = make_filler(hp) if hp + 2 < NHL // 2 else iter(())
                for ic in range(IC):
                    jt_max = 4 * ic + 3
                    pyA = ps_y.tile([HD + 1, 512], DT.float32, tag="y", name="pyA")
                    pyB = ps_y.tile([HD + 1, 512], DT.float32, tag="y", name="pyB")
                    pts = []
                    offs = []

                    def emit_pv(jt):
                        pt = pts[jt]
                        o = offs[jt]
                        nc.tensor.matmul(
                            pyA[:, o:512], v_sb[:, jt, hA, :], pt[:, o:512],
                            start=(jt == 0), stop=(jt == jt_max))
                        nc.tensor.matmul(
                            pyB[:, o:512], v_sb[:, jt, hB, :],
                            pt[:, 512 + o:1024],
                            start=(jt == 0), stop=(jt == jt_max))

                    for jt in range(jt_max + 1):
                        d = jt - 4 * ic
                        off = 128 * d if d > 0 else 0
                        ps = mm_tile()
                        isl = slice(ic * 512 + off, (ic + 1) * 512)
                        nc.tensor.matmul(
                            ps[:, off:512],
                            qkT_bf[0:HD, kt_i, jt * P:(jt + 1) * P],
                            qkT_bf[0:HD, qt, isl],
                            start=True, stop=True)
                        nc.tensor.matmul(
                            ps[:, 512 + off:1024],
                            qkT_bf[HD:P, kt_i, jt * P:(jt + 1) * P],
                            qkT_bf[HD:P, qt, isl],
                            start=True, stop=True)
                        ps2 = ps[:].rearrange("p (g x) -> p g x", g=2)
                        pt = pt_pool.tile([P, 1024], DT.bfloat16, tag="pt")
                        pt2 = pt[:].rearrange("p (g x) -> p g x", g=2)
                        if d >= 0:
                            # mask only the diagonal 128-col sub-block
                            nc.vector.tensor_tensor(
                                ps2[:, :, off:off + P], ps2[:, :, off:off + P],
                                masks[:, 0:1, :].to_broadcast((P, 2, P)),
                                ALU.add)
                            nc.scalar.activation(
                                pt2[:, :, off:512], ps2[:, :, off:512], AF.Exp)
                        else:
                            nc.scalar.activation(pt[:], ps[:], AF.Exp)
                        pts.append(pt)
                        offs.append(off)
                        if jt >= LAG:
                            emit_pv(jt - LAG)
                        next(filler, None)
                        next(filler, None)
                    for jt in range(max(0, jt_max + 1 - LAG), jt_max + 1):
                        emit_pv(jt)
                    # stash unnormalized z into yT (both heads at once); l rows
                    idxA, idxB = hA * IC + ic, hB * IC + ic
                    nc.vector.tensor_copy(
                        yT_bf[0:HD, hp, ic * 512:(ic + 1) * 512], pyA[0:HD, :])
                    nc.vector.tensor_copy(
                        yT_bf[HD:P, hp, ic * 512:(ic + 1) * 512], pyB[0:HD, :])
                    nc.vector.tensor_copy(
                        l_buf[32 * (idxA % 3):32 * (idxA % 3) + 1, idxA // 3, :],
                        pyA[HD:HD + 1, :])
                    nc.vector.tensor_copy(
                        l_buf[32 * (idxB % 3):32 * (idxB % 3) + 1, idxB // 3, :],
                        pyB[HD:HD + 1, :])
                    if ic == 0 and pending_tail is not None:
                        pending_tail()
                        pending_tail = None


                for _ in filler:
                    pass

                def make_tail(h, with_proj=False):
                    def tail():
                        # r = exp(-ln(l)): same ACT table set, no swap
                        c0 = (8 * h) // 3
                        ncol = (8 * h + 7) // 3 - c0 + 1
                        nc.scalar.activation(
                            ust[:, 0:ncol, :], l_buf[:, c0:c0 + ncol, :],
                            AF.Ln)
                        nc.scalar.activation(
                            r_bf[:, c0:c0 + ncol, :], ust[:, 0:ncol, :],
                            AF.Exp, scale=-1.0)
                        thA, thB = 2 * h, 2 * h + 1
                        for tic in range(IC):
                            idxA, idxB = thA * IC + tic, thB * IC + tic
                            pbt = qk_tile()
                            bA, bB = 32 * (idxA % 3), 32 * (idxB % 3)
                            nc.tensor.matmul(
                                pbt[0:HD, 0:512], ones1[bA:bA + 1, 0:HD],
                                r_bf[bA:bA + 1, idxA // 3, :],
                                start=True, stop=True)
                            nc.tensor.matmul(
                                pbt[HD:P, 0:512], ones1[bB:bB + 1, 0:HD],
                                r_bf[bB:bB + 1, idxB // 3, :],
                                start=True, stop=True, tile_position=(bB, HD))
                            ysl = yT_bf[:, h, tic * 512:(tic + 1) * 512]
                            nc.vector.tensor_mul(ysl, ysl, pbt[:])
                            nc.vector.tensor_scalar_add(
                                ysl, ysl, bv_sb[:, h:h + 1])
                            if with_proj:
                                for tt in range(4 * tic, 4 * tic + 4):
                                    proj_tt(tt)
                    return tail
                pending_tail = make_tail(hp, with_proj=(hp == NHL // 2 - 1))

            pending_tail()
    if split_waits:
        split_multi_waits(nc)
    return nc


_PROGRAM = None


def _get_program():
    global _PROGRAM
    if _PROGRAM is None:
        _PROGRAM = build_program()
    return _PROGRAM


def _make_in_maps(x, W_attn, b_attn, W_proj):
    x = np.asarray(x, dtype=np.float32)
    W_attn = np.asarray(W_attn, dtype=np.float32)
    b_attn = np.asarray(b_attn, dtype=np.float32)
    W_proj = np.asarray(W_proj, dtype=np.float32)
    in_maps = []
    for c in range(8):
        b, g = divmod(c, 2)
        sl = slice(CL * g, CL * (g + 1))
        wq = W_attn[:, 0:1024][:, sl]
        wk = W_attn[:, 1024:2048][:, sl]
        wv = W_attn[:, 2048:3072][:, sl]
        bq = b_attn[0:1024][sl]
        bk = b_attn[1024:2048][sl]
        bv = b_attn[2048:3072][sl]
        in_maps.append({
            "x": np.ascontiguousarray(x[b]),
            "wqk": np.ascontiguousarray(np.concatenate([wq, wk], axis=1)),
            "wv": np.ascontiguousarray(wv),
            "bqk": np.ascontiguousarray(np.concatenate([bq, bk])),
            "bv": np.ascontiguousarray(bv),
            "wp": np.ascontiguousarray(W_proj[sl]),
        })
    return in_maps


def kernel(x, W_attn, b_attn, W_proj, b_proj, _trace_dir=None):
    nc = _get_program()
    in_maps = _make_in_maps(x, W_attn, b_attn, W_proj)
    kwargs = {}
    if _trace_dir is not None:
        kwargs = dict(trace=True, tmpdir=_trace_dir)
    res = run_bass_kernel_spmd(nc, in_maps, core_ids=list(range(8)), **kwargs)
    b_proj = np.asarray(b_proj, dtype=np.float32)
    out = np.empty((4, T, COUT), dtype=np.float32)
    for b in range(4):
        out[b] = res.results[2 * b]["out"] + res.results[2 * b + 1]["out"] + b_proj
    if _trace_dir is not None:
        kernel._last_exec_time_ns = res.exec_time_ns
        kernel._last_results = res
    return out



# revision 25
# speedup vs baseline: 1.0480x; 1.0097x over previous
"""Causal self-attention kernel for 8 Trainium2 NeuronCores.

Sharding: core c -> (batch b = c // 2, head-group g = c % 2).
Each core computes attention for its batch over its 8 heads and a partial
output projection; the host sums the two head-group partials per batch and
adds b_proj.

Schedule (software-pipelined around the scalar-engine softmax exps, which
can only run on ACT and total ~160us/core):
  - prologue: x tiles DMA'd (issue spread across SP+ACT queues), cast to
    bf16 on DVE, transposed on the PE (bf16, 8 transposes into one bitcast
    psum tile + one wide copy-out); v for all heads and q/k for head pairs
    0-1 (W-stationary chains, DVE bias fold).
  - attention per head pair: both heads share the 128-wide S matmuls via
    PE row-group tiling (K=64 pairs run concurrently); causal mask is an
    additive NEG add restricted to the diagonal 128-col sub-block; PV lags
    S by LAG j-tiles and skips the zeroed sub-diagonal region; q/k chains
    for later head pairs are woven between S blocks at single-matmul
    granularity to keep the PE dense (HAM stays at K=8/8).
  - per-hp tail: 1/l = exp(-ln(l)) on ACT (Ln+Exp share one table set, so
    no ACT table swap mid-kernel), broadcast via a K=1 bf16 PE matmul,
    normalize+bias on DVE; each tail is emitted one chunk into the next
    hp's attention to avoid tensor-FIFO stalls; the last tail interleaves
    the output projection per 512-column chunk.

Reference shapes: x [4, 2048, 1024], W_attn [1024, 3072], b_attn [3072],
W_proj [1024, 1024], b_proj [1024]; NH=16, HD=64.
"""

import numpy as np

import bass_rust
import concourse.bass as bass
import concourse.mybir as mybir
import concourse.tile as tile
from concourse.bass_utils import run_bass_kernel_spmd

DT = mybir.dt
AF = mybir.ActivationFunctionType
ALU = mybir.AluOpType

P = 128
T = 2048          # sequence length
CIN = 1024        # input channels
CL = 512          # local channels (8 heads x 64)
NHL = 8           # local heads
HD = 64
KT = CIN // P     # 8 contraction tiles for qkv
TT = T // P       # 16 t-tiles
IC = T // 512     # 4 i-chunks of 512
COUT = 1024       # proj output channels
SCALE = 1.0 / 8.0  # 1/sqrt(HD)
NEG = -30000.0    # additive causal mask (exp underflows to 0)


class PatchedTileContext(tile.TileContext):
    """Work around walrus's 1-sync-wait-per-Drain limit: split the final
    drain's waits across one Drain instruction per proc."""

    def _drain_and_barrier(self, tick_clock, wait_clock):
        ScopedClock = bass_rust.ScopedClock
        VectorClock = bass_rust.VectorClock
        ticks = eval(repr(tick_clock.global_clock).replace("VectorClock(", "").rstrip(")"))
        for p, t in [(p, t) for p, t in enumerate(ticks) if t > 0]:
            part = [0] * len(ticks)
            part[p] = t
            d = self.nc.sync.drain()
            wait_clock.add_sem_waits(d.ins, ScopedClock({None: VectorClock(part)}))
        self.nc.all_engine_barrier()
        popped = self.nc._tile_sem_poison_stack.pop()
        assert popped is self._sem_poison
        self.nc.clear_and_free_semaphores(list(self.sems.allocated().values()))
        self.nc.all_engine_barrier()


# Max sync-waits this walrus build encodes per instruction. SP pseudo-DMA /
# CTRL instructions take a single wait; excess waits move onto NoOps that
# stall the same engine immediately before the instruction.
_MAX_WAITS = {}
_MAX_WAITS_DEFAULT = 1


def split_multi_waits(nc):
    for fn in nc.m.functions:
        for blk in fn.blocks:
            insts = blk.instructions
            out = []
            for inst in insts:
                si = getattr(inst, "sync_info", None)
                waits = list(si.on_wait) if si is not None and si.on_wait else []
                cap = _MAX_WAITS.get(str(inst.opcode), _MAX_WAITS_DEFAULT)
                if len(waits) > cap:
                    extra, keep = waits[:-cap], waits[-cap:]
                    for k, w in enumerate(extra):
                        nn = mybir.InstNoOp(name=f"{inst.name}-w{k}", ins=[], outs=[])
                        nn.engine = inst.engine
                        nn.sync_info = bass_rust.SyncInfo(on_wait=[w], on_update=[])
                        out.append(nn)
                    inst.sync_info = bass_rust.SyncInfo(
                        on_wait=keep, on_update=list(si.on_update or []))
                out.append(inst)
            blk.instructions = out


def act_reciprocal(nc, out, in_):
    """ACT-table reciprocal (bypasses the bass accuracy guard; tolerance here
    is loose enough)."""
    eng = nc.scalar
    inputs = [
        eng.lower_ap(in_),
        mybir.ImmediateValue(dtype=DT.float32, value=0.0),
        mybir.ImmediateValue(dtype=DT.float32, value=1.0),
        mybir.ImmediateValue(dtype=DT.float32, value=0.0),
    ]
    return eng.add_instruction(mybir.InstActivation(
        name=nc.get_next_instruction_name(),
        func=AF.Reciprocal,
        ins=inputs,
        outs=[eng.lower_ap(out)],
    ))


def build_program(split_waits=True):
    nc = bass.Bass()
    x_d = nc.dram_tensor("x", [T, CIN], DT.float32, kind="ExternalInput")
    wqk_d = nc.dram_tensor("wqk", [CIN, 2 * CL], DT.float32, kind="ExternalInput")
    wv_d = nc.dram_tensor("wv", [CIN, CL], DT.float32, kind="ExternalInput")
    bqk_d = nc.dram_tensor("bqk", [2 * CL], DT.float32, kind="ExternalInput")
    bv_d = nc.dram_tensor("bv", [CL], DT.float32, kind="ExternalInput")
    wp_d = nc.dram_tensor("wp", [CL, COUT], DT.float32, kind="ExternalInput")
    out_d = nc.dram_tensor("out", [T, COUT], DT.float32, kind="ExternalOutput")

    with PatchedTileContext(nc) as tc:
        with (
            tc.tile_pool(name="const", bufs=1) as const,
            tc.tile_pool(name="big", bufs=1) as big,
            tc.tile_pool(name="stage", bufs=2) as stage,
            tc.tile_pool(name="xs", bufs=2) as xs_pool,
            tc.tile_pool(name="xb", bufs=3) as xb_pool,
            tc.tile_pool(name="pt", bufs=10) as pt_pool,
            tc.tile_pool(name="ps_mm", bufs=2, space="PSUM") as ps_mm,
            tc.tile_pool(name="ps_qk", bufs=2, space="PSUM") as ps_qk,
            tc.tile_pool(name="ps_y", bufs=2, space="PSUM") as ps_y,
        ):
            # psum: mm [128,1024]x2 (4 banks) + qk [128,512]x2 + y x2 = 8
            def mm_tile():
                return ps_mm.tile([P, 1024], DT.float32, tag="mm", name="mmt")

            def qk_tile():
                return ps_qk.tile([P, 512], DT.float32, tag="qkp", name="qkp")

            # ---- constants ----
            ones1 = const.tile([65, P], DT.bfloat16, tag="ones1")
            nc.gpsimd.memset(ones1[:], 1.0)

            ident_bf = const.tile([P, P], DT.bfloat16, tag="ident")
            from concourse.masks import make_identity
            make_identity(nc, ident_bf[:])

            # causal mask for diagonal 128-col sub-blocks (d-independent):
            # mask[p, i] = 0 if i - p >= 0 else NEG
            masks = const.tile([P, 1, P], DT.float32, tag="masks")
            nc.gpsimd.memset(masks[:], 0.0)
            nc.gpsimd.affine_select(
                out=masks[:, 0, :],
                in_=masks[:, 0, :],
                compare_op=ALU.is_ge,
                fill=NEG,
                base=0,
                pattern=[[1, P]],
                channel_multiplier=-1,
            )

            # biases: bqk as [128, 8] per-partition layout (c_out on partitions)
            bqk_sb = const.tile([P, 2 * CL // P], DT.float32, tag="bqk")
            nc.sync.dma_start(bqk_sb[:], bqk_d.rearrange("(mt p) -> p mt", p=P))
            # bv_sb[64t+p, hp] = bv[64(2hp+t)+p]: head pair hp stacked on 128
            bv_sb = const.tile([P, NHL // 2], DT.float32, tag="bv")
            nc.sync.dma_start(
                bv_sb[:], bv_d.rearrange("(hp t p) -> (t p) hp", t=2, p=HD))

            # preload the exp ACT table during the prologue
            dummy = const.tile([1, 2], DT.float32, tag="dummy")
            nc.gpsimd.memset(dummy[:], 0.0)
            nc.scalar.activation(dummy[:], dummy[:], AF.Exp)

            # ---- x: DMA f32 -> DVE cast bf16 -> DMA-xbar transpose ----
            # xT[p, tt, ko, j] = x[128*tt + j, 128*ko + p]
            xT = big.tile([P, TT, KT, P], DT.bfloat16, tag="xT")
            x_r = x_d.rearrange("(tt p) c -> p tt c", p=P)
            qkT_bf = big.tile([P, KT, T], DT.bfloat16, tag="qkT_bf")
            v_sb = big.tile([P, TT, NHL, HD + 1], DT.bfloat16, tag="v_sb")
            nc.gpsimd.memset(v_sb[:, :, :, HD], 1.0)

            def load_x(tt):
                e1 = nc.sync if tt % 2 == 0 else nc.scalar
                xst = xs_pool.tile([P, CIN], DT.float32, tag="xstage")
                e1.dma_start(xst[:], x_r[:, tt, :])
                xbt = xb_pool.tile([P, CIN], DT.bfloat16, tag="xbstage")
                nc.vector.tensor_copy(xbt[:], xst[:])
                # 8 PE transposes into one psum tile (bf16 view), 1 copy out
                pst = mm_tile()
                pbf = pst[:].bitcast(DT.bfloat16)
                for ko in range(KT):
                    nc.tensor.transpose(
                        pbf[:, ko * P:(ko + 1) * P],
                        xbt[:, ko * P:(ko + 1) * P], ident_bf[:])
                nc.vector.tensor_copy(
                    xT[:, tt, :, :],
                    pbf[:, 0:CIN].rearrange("p (ko j) -> p ko j", ko=KT))

            for tt in range(4):
                load_x(tt)

            # ---- qkT = (x @ Wqk)^T in [c, t] layout; v in [t, c] layout ----
            # ---- weights: load fp32, cast to bf16 on DVE ----
            wqk_bf = big.tile([P, KT, 2 * CL], DT.bfloat16, tag="wqk_bf")
            wv_bf = big.tile([P, KT, CL], DT.bfloat16, tag="wv_bf")
            wqk_r = wqk_d.rearrange("(ko p) n -> p ko n", p=P)
            wv_r = wv_d.rearrange("(ko p) n -> p ko n", p=P)
            for ko in range(KT):
                stv = stage.tile([P, 2 * CL], DT.float32, tag="wstage", name="stv")[:, 0:CL]
                nc.scalar.dma_start(stv[:], wv_r[:, ko, :])
                nc.vector.tensor_copy(wv_bf[:, ko, :], stv[:])
                st = stage.tile([P, 2 * CL], DT.float32, tag="wstage", name="st")
                nc.scalar.dma_start(st[:], wqk_r[:, ko, :])
                nc.vector.tensor_copy(wqk_bf[:, ko, :], st[:])
            wp_bf = big.tile([P, CL // P, COUT], DT.bfloat16, tag="wp_bf")
            wp_r = wp_d.rearrange("(ko p) n -> p ko n", p=P)
            for ko in range(CL // P):
                stp = stage.tile([P, 2 * CL], DT.float32, tag="wstage", name="stp")[:, 0:COUT]
                nc.scalar.dma_start(stp[:], wp_r[:, ko, :])
                nc.gpsimd.tensor_copy(wp_bf[:, ko, :], stp[:])

            # qk matmul chain for one (c_out tile, t chunk); bias on DVE.
            # Generator form emits one instruction per next() so chains can
            # be woven between attention blocks at matmul granularity.
            def qk_chain_steps(mi, nic):
                pq = qk_tile()
                for ki in range(KT):
                    nc.tensor.matmul(
                        pq[:],
                        wqk_bf[:, ki, mi * P:(mi + 1) * P],
                        xT[:, 4 * nic:4 * nic + 4, ki, :],
                        start=(ki == 0), stop=(ki == KT - 1),
                    )
                    yield
                if mi < 4:
                    # q: (psum + bias) * SCALE
                    nc.vector.tensor_scalar(
                        qkT_bf[:, mi, nic * 512:(nic + 1) * 512],
                        pq[:], bqk_sb[:, mi:mi + 1], SCALE,
                        ALU.add, ALU.mult,
                    )
                else:
                    nc.vector.tensor_scalar_add(
                        qkT_bf[:, mi, nic * 512:(nic + 1) * 512],
                        pq[:], bqk_sb[:, mi:mi + 1],
                    )
                yield

            def qk_chain(mi, nic):
                for _ in qk_chain_steps(mi, nic):
                    pass

            # prologue: v for all heads + q,k for head-pair 0 only; the q,k
            # chains for hp+1 are interleaved into hp's attention below so the
            # PE stays busy while ACT runs the softmax exps.
            for nic in range(T // 512):
                if 4 * nic + 4 < TT:
                    for tt in range(4 * nic + 4, min(4 * nic + 8, TT)):
                        load_x(tt)
                # v = x @ Wv in [t, c] layout for the 4 t-tiles of this chunk
                for tt in range(4 * nic, 4 * nic + 4):
                    pv = qk_tile()
                    for ki in range(KT):
                        nc.tensor.matmul(
                            pv[:],
                            xT[:, tt, ki, :],
                            wv_bf[:, ki, :],
                            start=(ki == 0), stop=(ki == KT - 1),
                        )
                    nc.vector.tensor_copy(
                        v_sb[:, tt, :, 0:HD],
                        pv[:].rearrange("p (h e) -> p h e", h=NHL),
                    )
                for mi in (0, 4, 1, 5):
                    qk_chain(mi, nic)

            # ---- attention, head-pair packed, software-pipelined ----
            # Heads 2hp (partitions 0:64) and 2hp+1 (64:128) run as one
            # stream: S matmuls pack into row groups 0-1 / 2-3 concurrently,
            # one Exp covers both heads, PV lags LAG j-tiles behind S.
            # qk chains for hp+1 fill the PE while ACT exps hp; each hp's
            # normalize tail (1/l via ln+exp, same ACT table set) overlaps
            # the next hp's attention.
            yT_bf = big.tile([P, CL // P, T], DT.bfloat16, tag="yT_bf")
            out_r = out_d.rearrange("(tt p) c -> p tt c", p=P)

            def proj_tt(tt):
                pp = mm_tile()
                for oc in range(COUT // 512):
                    for ci in range(CL // P):
                        nc.tensor.matmul(
                            pp[:, oc * 512:(oc + 1) * 512],
                            yT_bf[:, ci, tt * P:(tt + 1) * P],
                            wp_bf[:, ci, oc * 512:(oc + 1) * 512],
                            start=(ci == 0), stop=(ci == CL // P - 1),
                        )
                ot = stage.tile([P, 2 * CL], DT.float32, tag="wstage",
                                name="ot")[:, 0:1024]
                if tt % 2 == 0:
                    nc.vector.tensor_copy(ot[:], pp[:])
                else:
                    nc.scalar.copy(ot[:], pp[:])
                nc.sync.dma_start(out_r[:, tt, :], ot[:])
            # l rows stored at partition bases {0,32,64} (matmul-rhs legal)
            l_buf = big.tile([65, 11, 512], DT.bfloat16, tag="l_buf")
            r_bf = big.tile([65, 11, 512], DT.bfloat16, tag="r_bf")
            ust = const.tile([65, 4, 512], DT.float32, tag="ust")
            LAG = 4
            pending_tail = None
            for hp in range(NHL // 2):
                hA, hB = 2 * hp, 2 * hp + 1
                qt, kt_i = hp, 4 + hp
                def make_filler(h):
                    for nic in range(4):
                        for mi in (h + 2, 6 + h):
                            yield from qk_chain_steps(mi, nic)
                filler = make_filler(hp) if hp + 2 < NHL // 2 else iter(())
                for ic in range(IC):
                    jt_max = 4 * ic + 3
                    pyA = ps_y.tile([HD + 1, 512], DT.float32, tag="y", name="pyA")
                    pyB = ps_y.tile([HD + 1, 512], DT.float32, tag="y", name="pyB")
                    pts = []
                    offs = []

                    def emit_pv(jt):
                        pt = pts[jt]
                        o = offs[jt]
                        nc.tensor.matmul(
                            pyA[:, o:512], v_sb[:, jt, hA, :], pt[:, o:512],
                            start=(jt == 0), stop=(jt == jt_max))
                        nc.tensor.matmul(
                            pyB[:, o:512], v_sb[:, jt, hB, :],
                            pt[:, 512 + o:1024],
                            start=(jt == 0), stop=(jt == jt_max))

                    for jt in range(jt_max + 1):
                        d = jt - 4 * ic
                        off = 128 * d if d > 0 else 0
                        ps = mm_tile()
                        isl = slice(ic * 512 + off, (ic + 1) * 512)
                        nc.tensor.matmul(
                            ps[:, off:512],
                            qkT_bf[0:HD, kt_i, jt * P:(jt + 1) * P],
                            qkT_bf[0:HD, qt, isl],
                            start=True, stop=True)
                        nc.tensor.matmul(
                            ps[:, 512 + off:1024],
                            qkT_bf[HD:P, kt_i, jt * P:(jt + 1) * P],
                            qkT_bf[HD:P, qt, isl],
                            start=True, stop=True)
                        ps2 = ps[:].rearrange("p (g x) -> p g x", g=2)
                        pt = pt_pool.tile([P, 1024], DT.bfloat16, tag="pt")
                        pt2 = pt[:].rearrange("p (g x) -> p g x", g=2)
                        if d >= 0:
                            # mask only the diagonal 128-col sub-block
                            nc.vector.tensor_tensor(
                                ps2[:, :, off:off + P], ps2[:, :, off:off + P],
                                masks[:, 0:1, :].to_broadcast((P, 2, P)),
                                ALU.add)
                            nc.scalar.activation(
                                pt2[:, :, off:512], ps2[:, :, off:512], AF.Exp)
                        else:
                            nc.scalar.activation(pt[:], ps[:], AF.Exp)
                        pts.append(pt)
                        offs.append(off)
                        if jt >= LAG:
                            emit_pv(jt - LAG)
                        next(filler, None)
                        next(filler, None)
                    for jt in range(max(0, jt_max + 1 - LAG), jt_max + 1):
                        emit_pv(jt)
                    # stash unnormalized z into yT (both heads at once); l rows
                    idxA, idxB = hA * IC + ic, hB * IC + ic
                    nc.vector.tensor_copy(
                        yT_bf[0:HD, hp, ic * 512:(ic + 1) * 512], pyA[0:HD, :])
                    nc.scalar.copy(
                        yT_bf[HD:P, hp, ic * 512:(ic + 1) * 512], pyB[0:HD, :])
                    nc.vector.tensor_copy(
                        l_buf[32 * (idxA % 3):32 * (idxA % 3) + 1, idxA // 3, :],
                        pyA[HD:HD + 1, :])
                    nc.vector.tensor_copy(
                        l_buf[32 * (idxB % 3):32 * (idxB % 3) + 1, idxB // 3, :],
                        pyB[HD:HD + 1, :])
                    if ic == 1 and pending_tail is not None:
                        pending_tail()
                        pending_tail = None


                for _ in filler:
                    pass

                def make_tail(h, with_proj=False):
                    def tail():
                        # r = exp(-ln(l)): same ACT table set, no swap
                        c0 = (8 * h) // 3
                        ncol = (8 * h + 7) // 3 - c0 + 1
                        nc.scalar.activation(
                            ust[:, 0:ncol, :], l_buf[:, c0:c0 + ncol, :],
                            AF.Ln)
                        nc.scalar.activation(
                            r_bf[:, c0:c0 + ncol, :], ust[:, 0:ncol, :],
                            AF.Exp, scale=-1.0)
                        thA, thB = 2 * h, 2 * h + 1
                        for tic in range(IC):
                            idxA, idxB = thA * IC + tic, thB * IC + tic
                            pbt = qk_tile()
                            bA, bB = 32 * (idxA % 3), 32 * (idxB % 3)
                            nc.tensor.matmul(
                                pbt[0:HD, 0:512], ones1[bA:bA + 1, 0:HD],
                                r_bf[bA:bA + 1, idxA // 3, :],
                                start=True, stop=True)
                            nc.tensor.matmul(
                                pbt[HD:P, 0:512], ones1[bB:bB + 1, 0:HD],
                                r_bf[bB:bB + 1, idxB // 3, :],
                                start=True, stop=True, tile_position=(bB, HD))
                            ysl = yT_bf[:, h, tic * 512:(tic + 1) * 512]
                            nc.vector.tensor_mul(ysl, ysl, pbt[:])
                            nc.vector.tensor_scalar_add(
                                ysl, ysl, bv_sb[:, h:h + 1])
                            if with_proj:
                                for tt in range(4 * tic, 4 * tic + 4):
                                    proj_tt(tt)
                    return tail
                pending_tail = make_tail(hp, with_proj=(hp == NHL // 2 - 1))

            pending_tail()
    if split_waits:
        split_multi_waits(nc)
    return nc


_PROGRAM = None


def _get_program():
    global _PROGRAM
    if _PROGRAM is None:
        _PROGRAM = build_program()
    return _PROGRAM


def _make_in_maps(x, W_attn, b_attn, W_proj):
    x = np.asarray(x, dtype=np.float32)
    W_attn = np.asarray(W_attn, dtype=np.float32)
    b_attn = np.asarray(b_attn, dtype=np.float32)
    W_proj = np.asarray(W_proj, dtype=np.float32)
    in_maps = []
    for c in range(8):
        b, g = divmod(c, 2)
        sl = slice(CL * g, CL * (g + 1))
        wq = W_attn[:, 0:1024][:, sl]
        wk = W_attn[:, 1024:2048][:, sl]
        wv = W_attn[:, 2048:3072][:, sl]
        bq = b_attn[0:1024][sl]
        bk = b_attn[1024:2048][sl]
        bv = b_attn[2048:3072][sl]
        in_maps.append({
            "x": np.ascontiguousarray(x[b]),
            "wqk": np.ascontiguousarray(np.concatenate([wq, wk], axis=1)),
            "wv": np.ascontiguousarray(wv),
            "bqk": np.ascontiguousarray(np.concatenate([bq, bk])),
            "bv": np.ascontiguousarray(bv),
            "wp": np.ascontiguousarray(W_proj[sl]),
        })
    return in_maps


def kernel(x, W_attn, b_attn, W_proj, b_proj, _trace_dir=None):
    nc = _get_program()
    in_maps = _make_in_maps(x, W_attn, b_attn, W_proj)
    kwargs = {}
    if _trace_dir is not None:
        kwargs = dict(trace=True, tmpdir=_trace_dir)
    res = run_bass_kernel_spmd(nc, in_maps, core_ids=list(range(8)), **kwargs)
    b_proj = np.asarray(b_proj, dtype=np.float32)
    out = np.empty((4, T, COUT), dtype=np.float32)
    for b in range(4):
        out[b] = res.results[2 * b]["out"] + res.results[2 * b + 1]["out"] + b_proj
    if _trace_dir is not None:
        kernel._last_exec_time_ns = res.exec_time_ns
        kernel._last_results = res
    return out

